# revision 1
# baseline (speedup 1.0000x reference)
"""AttnBlock (GroupNorm + single-head HWxHW attention + residual) on 8 trn2 cores.

Sharding: data-parallel over (batch, query-half): core i handles batch i//2,
query columns [ (i%2)*2048, (i%2+1)*2048 ).  The input for odd cores is
column-rotated on the host so every core's queries are columns 0:2048 of its
input (softmax over keys is permutation invariant, so k/v order doesn't
matter) -- this keeps the program SPMD (one NEFF for all 8 cores).

Device algorithm (per core, C=128 channels on partitions, N=4096 spatial):
  - GroupNorm stats: per-channel bn_stats/bn_aggr, then group (16-channel)
    reduction + broadcast via tiny mask matmuls on the PE.
  - q/k projections as [C,C]x[C,n] matmuls; v is produced directly
    TRANSPOSED (vT[m,c]) by using h-chunks as the stationary operand.
  - Scores are computed transposed: sT[m-tile, n-block] = k_tile^T . q_blk.
    exp() runs on the scalar engine straight out of PSUM over two banks at a
    time (no row-max subtraction needed: scores are O(10), fp32 exp is safe).
  - Softmax denominator (a cross-partition sum of the exp tiles) is split
    between engines to balance load: half the key tiles accumulate on the
    vector engine in SBUF, half accumulate on the PE into a PSUM row via
    ones-vector matmuls; the SBUF part is folded in with one more matmul.
  - PV: num[c, n] += vT_tile^T . pT_tile accumulated over 32 m-tiles in PSUM.
  - 1/den (fast Newton reciprocal on DVE) is broadcast across partitions by
    GPSIMD and fused into the PV-drain copy; the output projection then only
    needs a single residual add: out = x + wp.(num/den) [+ bias].

Host folding: gn_scale/gn_bias are folded into the q/k/v weights and biases;
the k bias is dropped entirely (additive per-query constant is softmax
invariant); the v bias is folded into the output projection bias since
softmax rows sum to 1.  Remaining biases are per-partition scalar adds, only
emitted when nonzero.

Matmuls run in float32r (fast fp32 mode, 1 PE cycle/row); all tiles feeding
them are produced pre-rounded (hardware requirement).
"""

import os
import sys
import types

if "/opt/trn_rl_repo" not in sys.path:
    sys.path.insert(0, "/opt/trn_rl_repo")

import numpy as np

B, C, H, W = 4, 128, 64, 64
N = H * W              # 4096 spatial positions
NQ = N // 2            # 2048 queries per core
NB = 512               # query block (columns per psum bank)
NBLK = NQ // NB        # 4 query blocks
MT = N // 128          # 32 key tiles
NCH = 4                # x/h chunking (1024 columns per chunk)
GROUPS = 8
GSIZE = C // GROUPS    # 16 channels per group
EPS = 1e-6
SCALE = float(C) ** -0.5
EXP_GRP = 2            # psum banks (512-wide matmuls) per exp instruction
DVE_DEN_G = 10         # exp groups whose denominator accumulates on DVE

# Set to False to run all big matmuls in full fp32 (4x slower, exact).
MM_FAST = os.environ.get("KERNEL_MM_FAST", "1") == "1"

LAST_RESULTS = None    # BassKernelResults of the most recent kernel() call


def _install_ntff_hook():
    """antenv.axon_hooks is missing from this container; inject it so
    run_bass_kernel_spmd(trace=True) can capture NTFF profiles."""
    if "antenv.axon_hooks" in sys.modules:
        return
    mod = types.ModuleType("antenv.axon_hooks")
    holder = [None]
    mod.set_axon_ntff_profile_hook = lambda h: holder.__setitem__(0, h)
    mod.get_axon_ntff_profile_hook = lambda: holder[0]
    sys.modules["antenv.axon_hooks"] = mod
    try:
        from trn_agent_boot.trn_boot import _ntff_profile_via_ctypes

        mod.set_axon_ntff_profile_hook(
            _ntff_profile_via_ctypes("/opt/axon/libaxon_pjrt.so")
        )
    except Exception:
        pass


_NC_CACHE = {}


def _build(mm_fast: bool, use_bq: bool, use_bp: bool):
    key = (mm_fast, use_bq, use_bp)
    if key in _NC_CACHE:
        return _NC_CACHE[key]

    import concourse.bacc as bacc
    import concourse.mybir as mybir
    import concourse.tile as tile

    f32 = mybir.dt.float32
    mmdt = mybir.dt.float32r if mm_fast else f32

    nc = bacc.Bacc("TRN2", target_bir_lowering=False, debug=False, num_devices=8)

    xp = nc.dram_tensor("xp", [C, N], f32, kind="ExternalInput")
    wqT_d = nc.dram_tensor("wqT", [C, C], f32, kind="ExternalInput")
    wkT_d = nc.dram_tensor("wkT", [C, C], f32, kind="ExternalInput")
    wvT_d = nc.dram_tensor("wvT", [C, C], f32, kind="ExternalInput")
    wpT_d = nc.dram_tensor("wpT", [C, C], f32, kind="ExternalInput")
    bq_d = nc.dram_tensor("bqe", [C, 1], f32, kind="ExternalInput")
    bp_d = nc.dram_tensor("bpe", [C, 1], f32, kind="ExternalInput")
    out_d = nc.dram_tensor("out", [C, NQ], f32, kind="ExternalOutput")

    # Group-mean reduction masks: gm averages a group's 16 channels (and the
    # spatial dim) into one row; gmT broadcasts group rows back to channels.
    gm_np = np.zeros((C, GROUPS), np.float32)
    gmT_np = np.zeros((GROUPS, C), np.float32)
    for ch in range(C):
        gm_np[ch, ch // GSIZE] = 1.0 / GSIZE
        gmT_np[ch // GSIZE, ch] = 1.0
    gm_d = nc.inline_tensor(gm_np, "gmask")
    gmT_d = nc.inline_tensor(gmT_np, "gmaskT")

    Exp = mybir.ActivationFunctionType.Exp
    Sqrt = mybir.ActivationFunctionType.Sqrt
    add_op = mybir.AluOpType.add
    sub_op = mybir.AluOpType.subtract
    mult_op = mybir.AluOpType.mult
    CHW = N // NCH  # 1024

    with tile.TileContext(nc) as tc:
        with (
            tc.tile_pool(name="big", bufs=1) as big,
            tc.tile_pool(name="wgt", bufs=1) as wgt,
            tc.tile_pool(name="ptile", bufs=8) as ptile,
            tc.tile_pool(name="small", bufs=2) as small,
            tc.tile_pool(name="ostage", bufs=3) as ostage,
            tc.tile_pool(name="ps_s", bufs=2, space="PSUM") as ps_s,
            tc.tile_pool(name="ps_pv", bufs=2, space="PSUM") as ps_pv,
            tc.tile_pool(name="ps_den", bufs=1, space="PSUM") as ps_den,
            tc.tile_pool(name="ps_m", bufs=1, space="PSUM") as ps_m,
        ):
            # --- load inputs: x first on both HWDGE rings (stats gate
            # everything), then weights/masks behind them ---
            xc = []
            for j in range(NCH):
                xj = big.tile([C, CHW], f32, tag=f"x{j}")
                eng = nc.sync if j % 2 == 0 else nc.scalar
                eng.dma_start(out=xj[:], in_=xp.ap()[:, j * CHW : (j + 1) * CHW])
                xc.append(xj)
            gm_sb = wgt.tile([C, GROUPS], f32, tag="gm")
            nc.sync.dma_start(out=gm_sb[:], in_=gm_d.ap())
            gmT_sb = wgt.tile([GROUPS, C], f32, tag="gmT")
            nc.sync.dma_start(out=gmT_sb[:], in_=gmT_d.ap())
            w_q0 = wgt.tile([C, C], f32, tag="wq0")
            nc.sync.dma_start(out=w_q0[:], in_=wqT_d.ap())
            w_k0 = wgt.tile([C, C], f32, tag="wk0")
            nc.scalar.dma_start(out=w_k0[:], in_=wkT_d.ap())
            w_v = wgt.tile([C, C], f32, tag="wv")
            nc.sync.dma_start(out=w_v[:], in_=wvT_d.ap())
            w_p0 = wgt.tile([C, C], f32, tag="wp0")
            nc.scalar.dma_start(out=w_p0[:], in_=wpT_d.ap())
            if use_bq:
                bq_sb = wgt.tile([C, 1], f32, tag="bq")
                nc.sync.dma_start(out=bq_sb[:], in_=bq_d.ap())
            if use_bp:
                bp_sb = wgt.tile([C, 1], f32, tag="bp")
                nc.sync.dma_start(out=bp_sb[:], in_=bp_d.ap())
            ones0 = wgt.tile([C, 1], f32, tag="ones0")
            nc.vector.memset(ones0[:], 1.0)
            eps_sb = wgt.tile([C, 1], f32, tag="eps")
            nc.vector.memset(eps_sb[:], EPS)
            # dummy activations force both ACT table sets to load now,
            # overlapped with the x DMAs, instead of mid-pipeline
            warm = wgt.tile([1, 1], f32, tag="warm")
            nc.scalar.activation(out=warm[:], in_=eps_sb[0:1, :], func=Sqrt)

            # --- GroupNorm statistics ---
            stats = small.tile([C, 8, 6], f32, tag="stats")
            for j in range(8):
                nc.vector.bn_stats(
                    out=stats[:, j, :],
                    in_=xc[j // 2][:, (j % 2) * 512 : (j % 2) * 512 + 512],
                )
            mv = small.tile([C, 2], f32, tag="mv")
            nc.vector.bn_aggr(out=mv[:], in_=stats[:])
            # t2 = per-channel [sum(x), sum(x^2)]; gm then averages over the group
            t2 = small.tile([C, 2], f32, tag="t2")
            nc.vector.tensor_copy(out=t2[:, 0:1], in_=mv[:, 0:1])
            nc.vector.tensor_tensor(t2[:, 1:2], mv[:, 0:1], mv[:, 0:1], mult_op)
            nc.vector.tensor_tensor(t2[:, 1:2], t2[:, 1:2], mv[:, 1:2], add_op)
            psg = ps_m.tile([GROUPS, 2], f32, tag="m")
            nc.tensor.matmul(psg[:], lhsT=gm_sb[:], rhs=t2[:], start=True, stop=True)
            g2 = small.tile([GROUPS, 2], f32, tag="g2")
            nc.vector.tensor_copy(out=g2[:], in_=psg[:])
            psb = ps_m.tile([C, 2], f32, tag="m")
            nc.tensor.matmul(psb[:], lhsT=gmT_sb[:], rhs=g2[:], start=True, stop=True)
            # mu = E[x]; var = E[x^2] - mu^2 ; rstd = 1/sqrt(var+eps)
            mu = small.tile([C, 1], f32, tag="mu")
            nc.vector.tensor_copy(out=mu[:], in_=psb[:, 0:1])
            var = small.tile([C, 1], f32, tag="var")
            nc.vector.tensor_tensor(var[:], mu[:], mu[:], mult_op)
            nc.vector.tensor_tensor(var[:], psb[:, 1:2], var[:], sub_op)
            sd = small.tile([C, 1], f32, tag="sd")
            nc.scalar.activation(out=sd[:], in_=var[:], func=Sqrt, bias=eps_sb[:])
            rstd = small.tile([C, 1], f32, tag="rstd")
            nc.vector.reciprocal_approx_fast(out=rstd[:], in_=sd[:])

            # weights rounded to f32r first (tiny DVE ops)
            if mm_fast:
                w_q = wgt.tile([C, C], mmdt, tag="wq")
                nc.vector.tensor_copy(out=w_q[:], in_=w_q0[:])
                w_k = wgt.tile([C, C], mmdt, tag="wk")
                nc.vector.tensor_copy(out=w_k[:], in_=w_k0[:])
                w_p = wgt.tile([C, C], mmdt, tag="wp")
                nc.vector.tensor_copy(out=w_p[:], in_=w_p0[:])
                ones_sb = wgt.tile([C, 1], mmdt, tag="ones")
                nc.vector.tensor_copy(out=ones_sb[:], in_=ones0[:])
            else:
                w_q, w_k, w_p = w_q0, w_k0, w_p0
                ones_sb = ones0

            # h = (x - mu) * rstd, chunked, with each chunk's q/k projections
            # emitted immediately behind it so block-0 scores can start as
            # soon as chunk 0 clears the in-order DVE queue.
            hc = []
            qb = [None] * NBLK
            kc = [None] * 8
            for j in range(NCH):
                hj = big.tile([C, CHW], mmdt, tag=f"h{j}")
                nc.vector.tensor_scalar(
                    hj[:], xc[j][:], mu[:], rstd[:], op0=sub_op, op1=mult_op
                )
                hc.append(hj)
                for s in range(2):
                    col = 2 * j + s
                    hs = hj[:, s * 512 : (s + 1) * 512]
                    if j < 2:
                        psq = ps_s.tile(
                            [C, EXP_GRP, 512], f32, tag="s", name=f"psq{col}"
                        )
                        nc.tensor.matmul(
                            psq[:, 0, :], lhsT=w_q[:], rhs=hs, start=True, stop=True
                        )
                        qj = big.tile([C, NB], mmdt, tag=f"q{col}")
                        if use_bq:
                            nc.vector.tensor_scalar_add(qj[:], psq[:, 0, :], bq_sb[:])
                        else:
                            nc.vector.tensor_copy(out=qj[:], in_=psq[:, 0, :])
                        qb[col] = qj
                    psk = ps_s.tile(
                        [C, EXP_GRP, 512], f32, tag="s", name=f"psk{col}"
                    )
                    nc.tensor.matmul(
                        psk[:, 0, :], lhsT=w_k[:], rhs=hs, start=True, stop=True
                    )
                    kj = big.tile([C, 512], mmdt, tag=f"k{col}")
                    nc.vector.tensor_copy(out=kj[:], in_=psk[:, 0, :])
                    kc[col] = kj

            def hpart(lo, width):
                j = lo // CHW
                assert lo + width <= (j + 1) * CHW
                return hc[j][:, lo - j * CHW : lo - j * CHW + width]

            def kpart(mi):
                return kc[mi // 4][:, (mi % 4) * 128 : (mi % 4) * 128 + 128]

            vT_sb = big.tile([128, MT, C], mmdt, tag="vt")

            def emit_vt(mi):
                # v^T tile, emitted lazily inside attention block 0.  Uses the
                # ps_m slot only: strictly FIFO there (pso comes later in
                # program order), so no slot-hold deadlock is possible.
                psv = ps_m.tile([C, C], f32, tag="m", name=f"psv{mi}")
                nc.tensor.matmul(
                    psv[:],
                    lhsT=hpart(mi * 128, 128).bitcast(f32),
                    rhs=w_v[:],
                    start=True,
                    stop=True,
                )
                nc.vector.tensor_copy(out=vT_sb[:, mi, :], in_=psv[:])

            # --- attention over query blocks ---
            for jb in range(NBLK):
                qs = qb[jb][:]
                pv = ps_pv.tile([C, NB], f32, tag="pv")
                dn = ps_den.tile([1, NB], f32, tag="dn")
                dacc = ostage.tile([128, EXP_GRP, NB], mmdt, tag="dacc")
                NG = MT // EXP_GRP
                pts = [None] * NG
                # software-pipelined by one group: scores/exp for g are
                # emitted (and scheduled) ahead of group g-1's consumers so
                # the scalar engine never starves behind PV/den matmuls.
                for g in range(NG + 1):
                    if g < NG:
                        ss = ps_s.tile([128, EXP_GRP, NB], f32, tag="s")
                        for u in range(EXP_GRP):
                            mi = g * EXP_GRP + u
                            nc.tensor.matmul(
                                ss[:, u, :],
                                lhsT=kpart(mi),
                                rhs=qs,
                                start=True,
                                stop=True,
                            )
                        pt = ptile.tile([128, EXP_GRP, NB], mmdt, tag="pt")
                        nc.scalar.activation(
                            out=pt[:], in_=ss[:], func=Exp, scale=SCALE
                        )
                        pts[g] = pt
                        if jb == 0:
                            for u in range(EXP_GRP):
                                emit_vt(g * EXP_GRP + u)
                    if g == 0:
                        continue
                    c = g - 1
                    pt = pts[c]
                    pts[c] = None
                    for u in range(EXP_GRP):
                        mi = c * EXP_GRP + u
                        nc.tensor.matmul(
                            pv[:],
                            lhsT=vT_sb[:, mi, :],
                            rhs=pt[:, u, :],
                            start=(mi == 0),
                            stop=(mi == MT - 1),
                        )
                    ptf = pt.bitcast(f32)
                    if c < DVE_DEN_G:
                        # denominator partial on DVE (SBUF adds)
                        if c == 0:
                            nc.vector.tensor_copy(out=dacc[:], in_=ptf[:])
                        else:
                            nc.vector.tensor_tensor(dacc[:], dacc[:], ptf[:], add_op)
                    else:
                        # denominator partial on PE (cross-partition sum)
                        for u in range(EXP_GRP):
                            mi = c * EXP_GRP + u
                            nc.tensor.matmul(
                                dn[:],
                                lhsT=ones_sb[:],
                                rhs=pt[:, u, :],
                                start=(c == DVE_DEN_G and u == 0),
                                stop=False,
                            )
                dfold = ostage.tile([128, NB], mmdt, tag="dfold")
                nc.vector.tensor_tensor(dfold[:], dacc[:, 0, :], dacc[:, 1, :], add_op)
                nc.tensor.matmul(
                    dn[:], lhsT=ones_sb[:], rhs=dfold[:], start=False, stop=True
                )
                rden = small.tile([1, NB], f32, tag="rden")
                nc.vector.reciprocal_approx_fast(out=rden[:], in_=dn[:])
                rb = ostage.tile([128, NB], f32, tag="rb")
                nc.gpsimd.partition_broadcast(rb[:], rden[:])
                # normalize during the PV drain, then project and add residual
                hv = ostage.tile([C, NB], mmdt, tag="hv")
                nc.vector.tensor_tensor(hv[:], pv[:], rb[:], mult_op)
                pso = ps_m.tile([C, NB], f32, tag="m")
                nc.tensor.matmul(
                    pso[:], lhsT=w_p[:], rhs=hv[:], start=True, stop=True
                )
                o1 = ostage.tile([C, NB], f32, tag="o1")
                xblk = xc[jb // 2][:, (jb % 2) * 512 : (jb % 2) * 512 + 512]
                nc.vector.tensor_tensor(o1[:], pso[:], xblk, add_op)
                if use_bp:
                    nc.vector.tensor_scalar_add(o1[:], o1[:], bp_sb[:])
                nc.sync.dma_start(out=out_d[:, jb * NB : (jb + 1) * NB], in_=o1[:])

    nc.compile()
    _NC_CACHE[key] = nc
    return nc


def kernel(**inputs):
    global LAST_RESULTS
    _install_ntff_hook()
    from concourse.bass_utils import run_bass_kernel_spmd

    ins = {
        k: np.ascontiguousarray(np.asarray(v), dtype=np.float32)
        for k, v in inputs.items()
    }
    x = ins["x"]
    gs, gb = ins["gn_scale"], ins["gn_bias"]

    # Fold the GroupNorm affine into the q/k/v weights; pre-transpose all
    # weights into the [in_channel, out_channel] layout the PE wants.
    wq_e = ins["wq"] * gs[None, :]
    wk_e = ins["wk"] * gs[None, :]
    wv_e = ins["wv"] * gs[None, :]
    wqT = np.ascontiguousarray(wq_e.T)
    wkT = np.ascontiguousarray(wk_e.T)
    wvT = np.ascontiguousarray(wv_e.T)
    wpT = np.ascontiguousarray(ins["wp"].T)
    bq_e = (ins["bq"] + ins["wq"] @ gb).reshape(C, 1)
    bv_e = ins["bv"] + ins["wv"] @ gb
    bp_e = (ins["bp"] + ins["wp"] @ bv_e).reshape(C, 1)
    use_bq = bool(np.any(bq_e))
    use_bp = bool(np.any(bp_e))

    nc = _build(MM_FAST, use_bq, use_bp)

    in_maps = []
    for core in range(8):
        b, half = core // 2, core % 2
        xb = x[b].reshape(C, N)
        if half == 1:
            xb = np.concatenate([xb[:, NQ:], xb[:, :NQ]], axis=1)
        in_maps.append(
            {
                "xp": np.ascontiguousarray(xb),
                "wqT": wqT,
                "wkT": wkT,
                "wvT": wvT,
                "wpT": wpT,
                "bqe": bq_e,
                "bpe": bp_e,
            }
        )

    trace = os.environ.get("KERNEL_TRACE", "0") == "1"
    res = run_bass_kernel_spmd(nc, in_maps, core_ids=list(range(8)), trace=trace)
    LAST_RESULTS = res

    out = np.empty((B, C, N), np.float32)
    for core in range(8):
        b, half = core // 2, core % 2
        out[b, :, half * NQ : (half + 1) * NQ] = res.results[core]["out"]
    return out.reshape(B, C, H, W)



# revision 3
# speedup vs baseline: 1.1586x; 1.1586x over previous
"""AttnBlock (GroupNorm + single-head HWxHW attention + residual) on 8 trn2 cores.

Sharding: data-parallel over (batch, query-half): core i handles batch i//2,
query columns [ (i%2)*2048, (i%2+1)*2048 ).  The input for odd cores is
column-rotated on the host so every core's queries are columns 0:2048 of its
input (softmax over keys is permutation invariant, so k/v order doesn't
matter) -- this keeps the program SPMD (one NEFF for all 8 cores).

Device algorithm (per core, C=128 channels on partitions, N=4096 spatial):
  - GroupNorm stats from bn_stats/bn_aggr on raw x chunks as they arrive;
    group (16-channel) reduction + broadcast via tiny mask matmuls on the PE.
  - rstd = exp(-0.5*ln(var+eps)) on the scalar engine: Ln and Exp live in the
    same activation table set, so the kernel needs exactly one ACT table load
    (warmed during the DMA prologue) -- no mid-pipeline table swap.
  - mu/rstd are folded into the weights on device (wq' = wqT*rstd etc., all
    cast to bf16), so q/k/v matmuls stream a bf16 copy of x directly; the h
    tensor is never materialized.  The -W'mu corrections: dropped for k
    (softmax-invariant), applied to q as a per-partition bias during the
    PSUM->SBUF copy, and folded into an output-projection bias for v.
  - Scores are computed transposed: sT[m-tile, n-block] = k_tile^T . q_blk in
    bf16 (fp32 PSUM).  exp() runs on the scalar engine over two PSUM banks at
    a time, writing bf16 tiles (no row-max: scores are O(5), fp32-safe).
  - Softmax denominator: the first DVE_DEN_G exp groups accumulate on the DVE
    in bf16 (2x mode), the rest on the PE into a [128,512] PSUM tile via
    all-ones [128,128] matmuls (every output row = den), so the reciprocal is
    produced already broadcast across partitions -- no gpsimd broadcast.
  - PV: num[c, n] += vT_tile^T . pT_tile accumulated over 32 m-tiles in PSUM;
    the drain multiplies by 1/den; the output projection then only needs one
    fused DVE op: out = (wp.(num/den) - obias) + x.
  - All projection matmuls (q/k/v/output-bias) that are not needed for the
    very first score group are emitted as per-group "fillers" inside block 0
    so the in-order PE/DVE queues never delay the first exp.

Host folding: gn_scale/gn_bias fold into the q/k/v weights and biases; the k
bias is dropped (softmax invariant); the v bias folds into the output bias.
"""

import os
import sys
import types

if "/opt/trn_rl_repo" not in sys.path:
    sys.path.insert(0, "/opt/trn_rl_repo")

import numpy as np

B, C, H, W = 4, 128, 64, 64
N = H * W              # 4096 spatial positions
NQ = N // 2            # 2048 queries per core
NB = 512               # query block (columns per psum bank)
NBLK = NQ // NB        # 4 query blocks
MT = N // 128          # 32 key tiles
NCH = 4                # x chunking (1024 columns per chunk)
CHW = N // NCH         # 1024
GROUPS = 8
GSIZE = C // GROUPS    # 16 channels per group
EPS = 1e-6
SCALE = float(C) ** -0.5
EXP_GRP = 2            # psum banks (512-wide matmuls) per exp instruction
NG = MT // EXP_GRP     # 16 exp groups per block
DVE_DEN_G = 12         # exp groups whose denominator accumulates on DVE

LAST_RESULTS = None    # BassKernelResults of the most recent kernel() call


def _install_ntff_hook():
    """antenv.axon_hooks is missing from this container; inject it so
    run_bass_kernel_spmd(trace=True) can capture NTFF profiles."""
    if "antenv.axon_hooks" in sys.modules:
        return
    mod = types.ModuleType("antenv.axon_hooks")
    holder = [None]
    mod.set_axon_ntff_profile_hook = lambda h: holder.__setitem__(0, h)
    mod.get_axon_ntff_profile_hook = lambda: holder[0]
    sys.modules["antenv.axon_hooks"] = mod
    try:
        from trn_agent_boot.trn_boot import _ntff_profile_via_ctypes

        mod.set_axon_ntff_profile_hook(
            _ntff_profile_via_ctypes("/opt/axon/libaxon_pjrt.so")
        )
    except Exception:
        pass


_NC_CACHE = {}


def _build(use_bq: bool, use_bp: bool):
    key = (use_bq, use_bp)
    if key in _NC_CACHE:
        return _NC_CACHE[key]

    import concourse.bacc as bacc
    import concourse.mybir as mybir
    import concourse.tile as tile

    f32 = mybir.dt.float32
    bf16 = mybir.dt.bfloat16

    nc = bacc.Bacc("TRN2", target_bir_lowering=False, debug=False, num_devices=8)

    xp = nc.dram_tensor("xp", [C, N], f32, kind="ExternalInput")
    wqT_d = nc.dram_tensor("wqT", [C, C], f32, kind="ExternalInput")
    wkT_d = nc.dram_tensor("wkT", [C, C], f32, kind="ExternalInput")
    wvT_d = nc.dram_tensor("wvT", [C, C], f32, kind="ExternalInput")
    wpT_d = nc.dram_tensor("wpT", [C, C], f32, kind="ExternalInput")
    bq_d = nc.dram_tensor("bqe", [C, 1], f32, kind="ExternalInput")
    bp_d = nc.dram_tensor("bpe", [C, 1], f32, kind="ExternalInput")
    out_d = nc.dram_tensor("out", [C, NQ], f32, kind="ExternalOutput")

    # Group-mean reduction masks: gm averages a group's 16 channels into one
    # row; gmT broadcasts group rows back to channels.
    gm_np = np.zeros((C, GROUPS), np.float32)
    gmT_np = np.zeros((GROUPS, C), np.float32)
    for ch in range(C):
        gm_np[ch, ch // GSIZE] = 1.0 / GSIZE
        gmT_np[ch // GSIZE, ch] = 1.0
    gm_d = nc.inline_tensor(gm_np, "gmask")
    gmT_d = nc.inline_tensor(gmT_np, "gmaskT")

    Exp = mybir.ActivationFunctionType.Exp
    Ln = mybir.ActivationFunctionType.Ln
    Copy = mybir.ActivationFunctionType.Copy
    add_op = mybir.AluOpType.add
    sub_op = mybir.AluOpType.subtract
    mult_op = mybir.AluOpType.mult

    with tile.TileContext(nc) as tc:
        with (
            tc.tile_pool(name="big", bufs=1) as big,
            tc.tile_pool(name="wgt", bufs=1) as wgt,
            tc.tile_pool(name="ptile", bufs=8) as ptile,
            tc.tile_pool(name="small", bufs=2) as small,
            tc.tile_pool(name="ostage", bufs=3) as ostage,
            tc.tile_pool(name="ps_s", bufs=2, space="PSUM") as ps_s,
            tc.tile_pool(name="ps_pv", bufs=2, space="PSUM") as ps_pv,
            tc.tile_pool(name="ps_den", bufs=1, space="PSUM") as ps_den,
            tc.tile_pool(name="ps_m", bufs=1, space="PSUM") as ps_m,
        ):
            # --- input DMAs: x chunks on the sync ring, weights/masks on the
            # scalar ring so they don't delay x ---
            xc = []
            for j in range(NCH):
                xj = big.tile([C, CHW], f32, tag=f"x{j}")
                nc.sync.dma_start(out=xj[:], in_=xp.ap()[:, j * CHW : (j + 1) * CHW])
                xc.append(xj)
            gm_sb = wgt.tile([C, GROUPS], f32, tag="gm")
            nc.scalar.dma_start(out=gm_sb[:], in_=gm_d.ap())
            gmT_sb = wgt.tile([GROUPS, C], f32, tag="gmT")
            nc.scalar.dma_start(out=gmT_sb[:], in_=gmT_d.ap())
            w_q0 = wgt.tile([C, C], f32, tag="wq0")
            nc.scalar.dma_start(out=w_q0[:], in_=wqT_d.ap())
            w_k0 = wgt.tile([C, C], f32, tag="wk0")
            nc.scalar.dma_start(out=w_k0[:], in_=wkT_d.ap())
            w_v0 = wgt.tile([C, C], f32, tag="wv0")
            nc.scalar.dma_start(out=w_v0[:], in_=wvT_d.ap())
            w_p0 = wgt.tile([C, C], f32, tag="wp0")
            nc.scalar.dma_start(out=w_p0[:], in_=wpT_d.ap())
            if use_bq:
                bqe_sb = wgt.tile([C, 1], f32, tag="bqe")
                nc.scalar.dma_start(out=bqe_sb[:], in_=bq_d.ap())
            if use_bp:
                bpe_sb = wgt.tile([C, 1], f32, tag="bpe")
                nc.scalar.dma_start(out=bpe_sb[:], in_=bp_d.ap())

            eps_sb = wgt.tile([C, 1], f32, tag="eps")
            nc.vector.memset(eps_sb[:], EPS)
            ones_bf = wgt.tile([C, C], bf16, tag="ones")
            nc.gpsimd.memset(ones_bf[:], 1.0)
            # dummy activation forces the ln/exp ACT table to load during the
            # x DMA instead of mid-pipeline (Ln, Exp, Copy share one set)
            warm = wgt.tile([1, 1], f32, tag="warm")
            nc.scalar.activation(out=warm[:], in_=eps_sb[0:1, :], func=Exp)

            # --- GroupNorm statistics (DVE, chunk-wise as x arrives) and
            # bf16 x copies (gpsimd, off the DVE critical path) ---
            stats = small.tile([C, 8, 6], f32, tag="stats")
            xb = []
            for j in range(NCH):
                for s in range(2):
                    nc.vector.bn_stats(
                        out=stats[:, 2 * j + s, :],
                        in_=xc[j][:, s * 512 : s * 512 + 512],
                    )
                xbj = big.tile([C, CHW], bf16, tag=f"xb{j}")
                nc.gpsimd.tensor_copy(out=xbj[:], in_=xc[j][:])
                xb.append(xbj)
            mv = small.tile([C, 2], f32, tag="mv")
            nc.vector.bn_aggr(out=mv[:], in_=stats[:])
            # t2 = per-channel [sum(x), sum(x^2)]; gm then averages the group
            t2 = small.tile([C, 2], f32, tag="t2")
            nc.vector.tensor_copy(out=t2[:, 0:1], in_=mv[:, 0:1])
            nc.vector.tensor_tensor(t2[:, 1:2], mv[:, 0:1], mv[:, 0:1], mult_op)
            nc.vector.tensor_tensor(t2[:, 1:2], t2[:, 1:2], mv[:, 1:2], add_op)
            psg = ps_m.tile([GROUPS, 2], f32, tag="m")
            nc.tensor.matmul(psg[:], lhsT=gm_sb[:], rhs=t2[:], start=True, stop=True)
            g2 = small.tile([GROUPS, 2], f32, tag="g2")
            nc.vector.tensor_copy(out=g2[:], in_=psg[:])
            psb = ps_m.tile([C, 2], f32, tag="m")
            nc.tensor.matmul(psb[:], lhsT=gmT_sb[:], rhs=g2[:], start=True, stop=True)
            # mu = E[x]; var = E[x^2] - mu^2 ; rstd = exp(-0.5*ln(var+eps))
            mu = small.tile([C, 1], f32, tag="mu")
            nc.vector.tensor_copy(out=mu[:], in_=psb[:, 0:1])
            var = small.tile([C, 1], f32, tag="var")
            nc.vector.tensor_tensor(var[:], mu[:], mu[:], mult_op)
            nc.vector.tensor_tensor(var[:], psb[:, 1:2], var[:], sub_op)
            lnv = small.tile([C, 1], f32, tag="lnv")
            nc.scalar.activation(out=lnv[:], in_=var[:], func=Ln, bias=eps_sb[:])
            rstd = small.tile([C, 1], f32, tag="rstd")
            nc.scalar.activation(out=rstd[:], in_=lnv[:], func=Exp, scale=-0.5)

            # fold rstd into the bf16 weights (per-in-channel scale)
            w_q = wgt.tile([C, C], bf16, tag="wq")
            nc.vector.tensor_scalar(w_q[:], w_q0[:], rstd[:], None, op0=mult_op)
            w_k = wgt.tile([C, C], bf16, tag="wk")
            nc.vector.tensor_scalar(w_k[:], w_k0[:], rstd[:], None, op0=mult_op)
            w_v = wgt.tile([C, C], bf16, tag="wv")
            nc.vector.tensor_scalar(w_v[:], w_v0[:], rstd[:], None, op0=mult_op)
            w_p = wgt.tile([C, C], bf16, tag="wp")
            nc.vector.tensor_copy(out=w_p[:], in_=w_p0[:])
            mub = small.tile([C, 1], bf16, tag="mub")
            nc.vector.tensor_copy(out=mub[:], in_=mu[:])

            def xbpart(lo, width):
                j = lo // CHW
                assert lo + width <= (j + 1) * CHW
                return xb[j][:, lo - j * CHW : lo - j * CHW + width]

            kc = [None] * 8

            def kpart(mi):
                return kc[mi // 4][:, (mi % 4) * 128 : (mi % 4) * 128 + 128]

            qb = [None] * NBLK
            vT_sb = big.tile([128, MT, C], bf16, tag="vt")
            bqs = small.tile([C, 1], f32, tag="bqs")
            bvb = small.tile([C, 1], bf16, tag="bvb")
            obs = small.tile([C, 1], f32, tag="obs")

            # --- critical-path projections: q block 0 and k slices 0..2 go
            # through the ps_s pool; everything else is a block-0 filler ---
            psq0 = ps_s.tile([C, EXP_GRP, NB], f32, tag="s", name="psq0")
            nc.tensor.matmul(
                psq0[:, 0, :], lhsT=w_q[:], rhs=xbpart(0, NB), start=True, stop=True
            )
            psk0 = ps_s.tile([C, EXP_GRP, NB], f32, tag="s", name="psk0")
            nc.tensor.matmul(
                psk0[:, 0, :], lhsT=w_k[:], rhs=xbpart(0, NB), start=True, stop=True
            )
            # q-bias correction -Wq'.mu (reuses the wq stationary)
            psbq = ps_m.tile([C, 1], f32, tag="m")
            nc.tensor.matmul(psbq[:], lhsT=w_q[:], rhs=mub[:], start=True, stop=True)
            psk1 = ps_s.tile([C, EXP_GRP, NB], f32, tag="s", name="psk1")
            nc.tensor.matmul(
                psk1[:, 0, :], lhsT=w_k[:], rhs=xbpart(NB, NB), start=True, stop=True
            )
            # v-bias chain part 1: Wv'.mu
            psbv = ps_m.tile([C, 1], f32, tag="m")
            nc.tensor.matmul(psbv[:], lhsT=w_v[:], rhs=mub[:], start=True, stop=True)
            psk2 = ps_s.tile([C, EXP_GRP, NB], f32, tag="s", name="psk2")
            nc.tensor.matmul(
                psk2[:, 0, :], lhsT=w_k[:], rhs=xbpart(2 * NB, NB), start=True, stop=True
            )

            nc.vector.tensor_copy(out=bqs[:], in_=psbq[:])
            nc.vector.tensor_copy(out=bvb[:], in_=psbv[:])
            q0 = big.tile([C, NB], bf16, tag="q0")
            if use_bq:
                nc.vector.tensor_scalar(
                    q0[:], psq0[:, 0, :], bqs[:], bqe_sb[:], op0=sub_op, op1=add_op
                )
            else:
                nc.vector.tensor_scalar(q0[:], psq0[:, 0, :], bqs[:], None, op0=sub_op)
            qb[0] = q0
            # k slices 0,1 copied on the scalar engine (idle until first exp);
            # slice 2 on DVE
            k0t = big.tile([C, NB], bf16, tag="k0")
            nc.scalar.activation(out=k0t[:], in_=psk0[:, 0, :], func=Copy)
            kc[0] = k0t
            k1t = big.tile([C, NB], bf16, tag="k1")
            nc.scalar.activation(out=k1t[:], in_=psk1[:, 0, :], func=Copy)
            kc[1] = k1t
            k2t = big.tile([C, NB], bf16, tag="k2")
            nc.vector.tensor_copy(out=k2t[:], in_=psk2[:, 0, :])
            kc[2] = k2t

            # --- filler emitters (block-0 interleaved projections) ---
            def emit_k(s):
                psk = ps_m.tile([C, NB], f32, tag="m", name=f"psk{s}")
                nc.tensor.matmul(
                    psk[:], lhsT=w_k[:], rhs=xbpart(s * NB, NB), start=True, stop=True
                )
                kj = big.tile([C, NB], bf16, tag=f"k{s}")
                nc.vector.tensor_copy(out=kj[:], in_=psk[:])
                kc[s] = kj

            def emit_q(jb):
                psq = ps_m.tile([C, NB], f32, tag="m", name=f"psq{jb}")
                nc.tensor.matmul(
                    psq[:], lhsT=w_q[:], rhs=xbpart(jb * NB, NB), start=True, stop=True
                )
                qj = big.tile([C, NB], bf16, tag=f"q{jb}")
                if use_bq:
                    nc.vector.tensor_scalar(
                        qj[:], psq[:], bqs[:], bqe_sb[:], op0=sub_op, op1=add_op
                    )
                else:
                    nc.vector.tensor_scalar(qj[:], psq[:], bqs[:], None, op0=sub_op)
                qb[jb] = qj

            def emit_vb(b):
                # 4 transposed v tiles into one psum bank, one DVE drain
                psv = ps_m.tile([128, 4, C], f32, tag="m", name=f"psv{b}")
                for t in range(4):
                    mi = 4 * b + t
                    nc.tensor.matmul(
                        psv[:, t, :],
                        lhsT=xbpart(mi * 128, 128),
                        rhs=w_v[:],
                        start=True,
                        stop=True,
                    )
                nc.vector.tensor_copy(out=vT_sb[:, 4 * b : 4 * b + 4, :], in_=psv[:])

            def emit_psob():
                # v-bias chain part 2: obias = Wp.(Wv'.mu)  [minus bpe if any]
                psob = ps_m.tile([C, 1], f32, tag="m")
                nc.tensor.matmul(psob[:], lhsT=w_p[:], rhs=bvb[:], start=True, stop=True)
                if use_bp:
                    nc.vector.tensor_tensor(obs[:], psob[:], bpe_sb[:], sub_op)
                else:
                    nc.vector.tensor_copy(out=obs[:], in_=psob[:])

            # filler schedule: (block, group) -> emitter.  k slice s must land
            # before score group 2s; v batch b before PV step 2b+1; q_jb before
            # block jb; psob before block 0's tail.
            fillers = {
                (0, 0): lambda: emit_vb(0),
                (0, 1): lambda: emit_vb(1),
                (0, 2): lambda: emit_k(3),
                (0, 3): lambda: emit_vb(2),
                (0, 4): lambda: emit_k(4),
                (0, 5): lambda: emit_vb(3),
                (0, 6): lambda: emit_q(1),
                (0, 7): lambda: emit_vb(4),
                (0, 8): lambda: emit_k(5),
                (0, 9): lambda: emit_vb(5),
                (0, 10): lambda: emit_k(6),
                (0, 11): lambda: emit_vb(6),
                (0, 12): lambda: emit_k(7),
                (0, 13): lambda: emit_vb(7),
                (0, 14): emit_psob,
                (1, 0): lambda: emit_q(2),
                (1, 1): lambda: emit_q(3),
            }

            # --- attention over query blocks ---
            for jb in range(NBLK):
                qs = qb[jb][:]
                pv = ps_pv.tile([C, NB], f32, tag="pv")
                dn = ps_den.tile([C, NB], f32, tag="dn")
                dacc = ostage.tile([128, EXP_GRP, NB], bf16, tag="dacc")
                dfold = ostage.tile([128, NB], bf16, tag="dfold")
                pts = [None] * NG
                # software-pipelined by one group: scores/exp for g are
                # emitted ahead of group g-1's consumers so the scalar engine
                # never starves behind PV/den matmuls.
                for g in range(NG + 1):
                    if g < NG:
                        ss = ps_s.tile([128, EXP_GRP, NB], f32, tag="s")
                        for u in range(EXP_GRP):
                            mi = g * EXP_GRP + u
                            nc.tensor.matmul(
                                ss[:, u, :],
                                lhsT=kpart(mi),
                                rhs=qs,
                                start=True,
                                stop=True,
                            )
                        pt = ptile.tile([128, EXP_GRP, NB], bf16, tag="pt")
                        nc.scalar.activation(
                            out=pt[:], in_=ss[:], func=Exp, scale=SCALE
                        )
                        pts[g] = pt
                        fill = fillers.get((jb, g))
                        if fill is not None:
                            fill()
                    if g == 0:
                        continue
                    c = g - 1
                    pt = pts[c]
                    pts[c] = None
                    for u in range(EXP_GRP):
                        mi = c * EXP_GRP + u
                        nc.tensor.matmul(
                            pv[:],
                            lhsT=vT_sb[:, mi, :],
                            rhs=pt[:, u, :],
                            start=(mi == 0),
                            stop=(mi == MT - 1),
                        )
                    if c < DVE_DEN_G:
                        # denominator partial on DVE (bf16 SBUF adds, 2x mode)
                        if c == 0:
                            nc.vector.tensor_copy(out=dacc[:], in_=pt[:])
                        else:
                            nc.vector.tensor_tensor(dacc[:], dacc[:], pt[:], add_op)
                        if c == DVE_DEN_G - 1:
                            nc.vector.tensor_tensor(
                                dfold[:], dacc[:, 0, :], dacc[:, 1, :], add_op
                            )
                    else:
                        # denominator partial on PE; the all-ones stationary
                        # writes den into every output partition, so the
                        # reciprocal needs no cross-partition broadcast
                        for u in range(EXP_GRP):
                            mi = c * EXP_GRP + u
                            nc.tensor.matmul(
                                dn[:],
                                lhsT=ones_bf[:],
                                rhs=pt[:, u, :],
                                start=(c == DVE_DEN_G and u == 0),
                                stop=False,
                            )
                nc.tensor.matmul(
                    dn[:], lhsT=ones_bf[:], rhs=dfold[:], start=False, stop=True
                )
                rden = ostage.tile([128, NB], f32, tag="rden")
                nc.vector.reciprocal_approx_fast(out=rden[:], in_=dn[:])
                # normalize during the PV drain, then project; the residual
                # and output bias fuse into one DVE op
                hv = ostage.tile([C, NB], bf16, tag="hv")
                nc.vector.tensor_tensor(hv[:], pv[:], rden[:], mult_op)
                pso = ps_m.tile([C, NB], f32, tag="m")
                nc.tensor.matmul(pso[:], lhsT=w_p[:], rhs=hv[:], start=True, stop=True)
                o1 = ostage.tile([C, NB], f32, tag="o1")
                xblk = xc[jb // 2][:, (jb % 2) * 512 : (jb % 2) * 512 + 512]
                nc.vector.scalar_tensor_tensor(
                    o1[:], pso[:], obs[:], xblk, op0=sub_op, op1=add_op
                )
                nc.sync.dma_start(out=out_d[:, jb * NB : (jb + 1) * NB], in_=o1[:])

    nc.compile()
    _NC_CACHE[key] = nc
    return nc


def kernel(**inputs):
    global LAST_RESULTS
    _install_ntff_hook()
    from concourse.bass_utils import run_bass_kernel_spmd

    ins = {
        k: np.ascontiguousarray(np.asarray(v), dtype=np.float32)
        for k, v in inputs.items()
    }
    x = ins["x"]
    gs, gb = ins["gn_scale"], ins["gn_bias"]

    # Fold the GroupNorm affine into the q/k/v weights; pre-transpose all
    # weights into the [in_channel, out_channel] layout the PE wants.
    wq_e = ins["wq"] * gs[None, :]
    wk_e = ins["wk"] * gs[None, :]
    wv_e = ins["wv"] * gs[None, :]
    wqT = np.ascontiguousarray(wq_e.T)
    wkT = np.ascontiguousarray(wk_e.T)
    wvT = np.ascontiguousarray(wv_e.T)
    wpT = np.ascontiguousarray(ins["wp"].T)
    bq_e = (ins["bq"] + ins["wq"] @ gb).reshape(C, 1)
    bv_e = ins["bv"] + ins["wv"] @ gb
    bp_e = (ins["bp"] + ins["wp"] @ bv_e).reshape(C, 1)
    use_bq = bool(np.any(bq_e))
    use_bp = bool(np.any(bp_e))

    nc = _build(use_bq, use_bp)

    in_maps = []
    for core in range(8):
        b, half = core // 2, core % 2
        xb = x[b].reshape(C, N)
        if half == 1:
            xb = np.concatenate([xb[:, NQ:], xb[:, :NQ]], axis=1)
        in_maps.append(
            {
                "xp": np.ascontiguousarray(xb),
                "wqT": wqT,
                "wkT": wkT,
                "wvT": wvT,
                "wpT": wpT,
                "bqe": bq_e,
                "bpe": bp_e,
            }
        )

    trace = os.environ.get("KERNEL_TRACE", "0") == "1"
    res = run_bass_kernel_spmd(nc, in_maps, core_ids=list(range(8)), trace=trace)
    LAST_RESULTS = res

    out = np.empty((B, C, N), np.float32)
    for core in range(8):
        b, half = core // 2, core % 2
        out[b, :, half * NQ : (half + 1) * NQ] = res.results[core]["out"]
    return out.reshape(B, C, H, W)


# revision 6
# speedup vs baseline: 1.2153x; 1.0490x over previous
"""AttnBlock (GroupNorm + single-head HWxHW attention + residual) on 8 trn2 cores.

Sharding: data-parallel over (batch, query-half): core i handles batch i//2,
query columns [ (i%2)*2048, (i%2+1)*2048 ).  The input for odd cores is
column-rotated on the host so every core's queries are columns 0:2048 of its
input (softmax over keys is permutation invariant, so k/v order doesn't
matter) -- this keeps the program SPMD (one NEFF for all 8 cores).

Device algorithm (per core, C=128 channels on partitions, N=4096 spatial):
  - x streams in 4 chunks alternating both HWDGE rings; GroupNorm stats run
    chunk-wise on the DVE as data arrives, interleaved with bf16 casts of x.
  - rstd = exp(-0.5*ln(var+eps)) on the scalar engine: Ln/Exp/Copy share one
    activation table set, so the kernel does exactly one ACT table load,
    warmed during the DMA prologue.  No mid-pipeline table swap.
  - mu/rstd fold into bf16 copies of the weights on device, so q/k/v matmuls
    stream the bf16 x directly (no h tensor).  The -W'mu corrections: dropped
    for k (softmax-invariant), applied to q as a per-partition bias in the
    PSUM->SBUF copy, folded into an output-projection bias for v.
  - While waiting for x, the PE runs a train of dummy matmuls to ramp the
    DVFS p-state so the projection/score matmuls start at full clock.
  - Scores are computed transposed: sT[m-tile, n-block] = k_tile^T . q_blk in
    bf16 (fp32 PSUM).  exp() runs on the scalar engine over two PSUM banks at
    a time, writing bf16 (no row-max: scores are O(5), fp32-safe).
  - Softmax denominator: the first DEN_SPLIT[jb] exp groups accumulate on the
    DVE in bf16 (2x mode), the rest on the PE into a [128,512] PSUM tile via
    all-ones [128,128] matmuls (every output row = den), so the reciprocal is
    already broadcast across partitions -- no gpsimd broadcast.
  - PV: num[c, n] += vT_tile^T . pT_tile accumulated over 32 m-tiles in PSUM;
    the drain multiplies by 1/den; the output projection needs only one fused
    DVE op: out = (wp.(num/den) - obias) + x.
  - Block tails (den fold / reciprocal / PV drain / projection / store) are
    deferred into the NEXT block's early iterations so the in-order PE queue
    never stalls the next block's score matmuls; projections not needed for
    the first score group are spread as per-group fillers across block 0.

Host folding: gn_scale/gn_bias fold into the q/k/v weights and biases; the k
bias is dropped (softmax invariant); the v bias folds into the output bias.
"""

import os
import sys
import types

if "/opt/trn_rl_repo" not in sys.path:
    sys.path.insert(0, "/opt/trn_rl_repo")

import numpy as np

B, C, H, W = 4, 128, 64, 64
N = H * W              # 4096 spatial positions
NQ = N // 2            # 2048 queries per core
NB = 512               # query block (columns per psum bank)
NBLK = NQ // NB        # 4 query blocks
MT = N // 128          # 32 key tiles
NCH = 4                # x chunking (1024 columns per chunk)
CHW = N // NCH         # 1024
GROUPS = 8
GSIZE = C // GROUPS    # 16 channels per group
EPS = 1e-6
SCALE = float(C) ** -0.5
EXP_GRP = 2            # psum banks (512-wide matmuls) per exp instruction
NG = MT // EXP_GRP     # 16 exp groups per block
DEN_SPLIT = [10, 16, 16, 12]  # per block: exp groups accumulated on DVE
N_WARM = 16            # dummy matmuls to ramp the PE p-state during x DMA

LAST_RESULTS = None    # BassKernelResults of the most recent kernel() call


def _install_ntff_hook():
    """antenv.axon_hooks is missing from this container; inject it so
    run_bass_kernel_spmd(trace=True) can capture NTFF profiles."""
    if "antenv.axon_hooks" in sys.modules:
        return
    mod = types.ModuleType("antenv.axon_hooks")
    holder = [None]
    mod.set_axon_ntff_profile_hook = lambda h: holder.__setitem__(0, h)
    mod.get_axon_ntff_profile_hook = lambda: holder[0]
    sys.modules["antenv.axon_hooks"] = mod
    try:
        from trn_agent_boot.trn_boot import _ntff_profile_via_ctypes

        mod.set_axon_ntff_profile_hook(
            _ntff_profile_via_ctypes("/opt/axon/libaxon_pjrt.so")
        )
    except Exception:
        pass


_NC_CACHE = {}


def _build(use_bq: bool, use_bp: bool):
    key = (use_bq, use_bp)
    if key in _NC_CACHE:
        return _NC_CACHE[key]

    import concourse.bacc as bacc
    import concourse.mybir as mybir
    import concourse.tile as tile

    f32 = mybir.dt.float32
    bf16 = mybir.dt.bfloat16

    nc = bacc.Bacc("TRN2", target_bir_lowering=False, debug=False, num_devices=8)

    xp = nc.dram_tensor("xp", [C, N], f32, kind="ExternalInput")
    wqT_d = nc.dram_tensor("wqT", [C, C], f32, kind="ExternalInput")
    wkT_d = nc.dram_tensor("wkT", [C, C], f32, kind="ExternalInput")
    wvT_d = nc.dram_tensor("wvT", [C, C], f32, kind="ExternalInput")
    wpT_d = nc.dram_tensor("wpT", [C, C], f32, kind="ExternalInput")
    bq_d = nc.dram_tensor("bqe", [C, 1], f32, kind="ExternalInput")
    bp_d = nc.dram_tensor("bpe", [C, 1], f32, kind="ExternalInput")
    out_d = nc.dram_tensor("out", [C, NQ], f32, kind="ExternalOutput")

    # Group-mean reduction masks: gm averages a group's 16 channels into one
    # row; gmT broadcasts group rows back to channels.
    gm_np = np.zeros((C, GROUPS), np.float32)
    gmT_np = np.zeros((GROUPS, C), np.float32)
    for ch in range(C):
        gm_np[ch, ch // GSIZE] = 1.0 / GSIZE
        gmT_np[ch // GSIZE, ch] = 1.0
    gm_d = nc.inline_tensor(gm_np, "gmask")
    gmT_d = nc.inline_tensor(gmT_np, "gmaskT")

    Exp = mybir.ActivationFunctionType.Exp
    Ln = mybir.ActivationFunctionType.Ln
    Copy = mybir.ActivationFunctionType.Copy
    add_op = mybir.AluOpType.add
    sub_op = mybir.AluOpType.subtract
    mult_op = mybir.AluOpType.mult

    with tile.TileContext(nc) as tc:
        with (
            tc.tile_pool(name="big", bufs=1) as big,
            tc.tile_pool(name="wgt", bufs=1) as wgt,
            tc.tile_pool(name="ptile", bufs=8) as ptile,
            tc.tile_pool(name="small", bufs=2) as small,
            tc.tile_pool(name="ostage", bufs=3) as ostage,
            tc.tile_pool(name="ps_s", bufs=2, space="PSUM") as ps_s,
            tc.tile_pool(name="ps_pv", bufs=2, space="PSUM") as ps_pv,
            tc.tile_pool(name="ps_den", bufs=1, space="PSUM") as ps_den,
            tc.tile_pool(name="ps_m", bufs=1, space="PSUM") as ps_m,
        ):
            # --- input DMAs: x chunks alternate both HWDGE rings (each ring
            # feeds its own DMA-engine subset), weights/masks follow x ---
            xc = []
            for j in range(NCH):
                xj = big.tile([C, CHW], f32, tag=f"x{j}")
                eng = nc.sync if j % 2 == 0 else nc.scalar
                eng.dma_start(out=xj[:], in_=xp.ap()[:, j * CHW : (j + 1) * CHW])
                xc.append(xj)
            gm_sb = wgt.tile([C, GROUPS], f32, tag="gm")
            nc.sync.dma_start(out=gm_sb[:], in_=gm_d.ap())
            gmT_sb = wgt.tile([GROUPS, C], f32, tag="gmT")
            nc.sync.dma_start(out=gmT_sb[:], in_=gmT_d.ap())
            w_q0 = wgt.tile([C, C], f32, tag="wq0")
            nc.scalar.dma_start(out=w_q0[:], in_=wqT_d.ap())
            w_k0 = wgt.tile([C, C], f32, tag="wk0")
            nc.sync.dma_start(out=w_k0[:], in_=wkT_d.ap())
            w_v0 = wgt.tile([C, C], f32, tag="wv0")
            nc.scalar.dma_start(out=w_v0[:], in_=wvT_d.ap())
            w_p0 = wgt.tile([C, C], f32, tag="wp0")
            nc.sync.dma_start(out=w_p0[:], in_=wpT_d.ap())
            if use_bq:
                bqe_sb = wgt.tile([C, 1], f32, tag="bqe")
                nc.scalar.dma_start(out=bqe_sb[:], in_=bq_d.ap())
            if use_bp:
                bpe_sb = wgt.tile([C, 1], f32, tag="bpe")
                nc.scalar.dma_start(out=bpe_sb[:], in_=bp_d.ap())

            eps_sb = wgt.tile([C, 1], f32, tag="eps")
            nc.vector.memset(eps_sb[:], EPS)
            ones_bf = wgt.tile([C, C], bf16, tag="ones")
            nc.gpsimd.memset(ones_bf[:], 1.0)
            # dummy Ln forces the ln/exp/copy ACT table to load during the
            # x DMA; it is the only table set this kernel ever needs
            warm = wgt.tile([1, 1], f32, tag="warm")
            nc.scalar.activation(out=warm[:], in_=eps_sb[0:1, :], func=Ln)

            # --- GroupNorm statistics + bf16 x casts (DVE, chunk-wise) ---
            stats = small.tile([C, 8, 6], f32, tag="stats")
            xb = []
            for j in range(NCH):
                for s in range(2):
                    nc.vector.bn_stats(
                        out=stats[:, 2 * j + s, :],
                        in_=xc[j][:, s * 512 : s * 512 + 512],
                    )
                if j < 3:
                    xbj = big.tile([C, CHW], bf16, tag=f"xb{j}")
                    nc.vector.tensor_copy(out=xbj[:], in_=xc[j][:])
                    xb.append(xbj)
            xb3 = big.tile([C, CHW], bf16, tag="xb3")  # cast deferred
            xb.append(xb3)

            # PE p-state warm-up: dummy matmuls into a throwaway psum bank
            # while the tail x chunks stream in (consumes nothing downstream)
            ps_warm = ps_m.tile([C, NB], f32, tag="m")
            for _ in range(N_WARM):
                nc.tensor.matmul(
                    ps_warm[:], lhsT=ones_bf[:], rhs=xb[0][:, 0:NB],
                    start=True, stop=True,
                )

            mv = small.tile([C, 2], f32, tag="mv")
            nc.vector.bn_aggr(out=mv[:], in_=stats[:])
            # t2 = per-channel [sum(x), sum(x^2)]; gm then averages the group
            t2 = small.tile([C, 2], f32, tag="t2")
            nc.vector.tensor_copy(out=t2[:, 0:1], in_=mv[:, 0:1])
            nc.vector.tensor_tensor(t2[:, 1:2], mv[:, 0:1], mv[:, 0:1], mult_op)
            nc.vector.tensor_tensor(t2[:, 1:2], t2[:, 1:2], mv[:, 1:2], add_op)
            psg = ps_m.tile([GROUPS, 2], f32, tag="m")
            nc.tensor.matmul(psg[:], lhsT=gm_sb[:], rhs=t2[:], start=True, stop=True)
            g2 = small.tile([GROUPS, 2], f32, tag="g2")
            nc.vector.tensor_copy(out=g2[:], in_=psg[:])
            psb = ps_m.tile([C, 2], f32, tag="m")
            nc.tensor.matmul(psb[:], lhsT=gmT_sb[:], rhs=g2[:], start=True, stop=True)
            # mu = E[x]; var = E[x^2] - mu^2 ; rstd = exp(-0.5*ln(var+eps))
            mu = small.tile([C, 1], f32, tag="mu")
            nc.vector.tensor_copy(out=mu[:], in_=psb[:, 0:1])
            mub = small.tile([C, 1], bf16, tag="mub")
            nc.vector.tensor_copy(out=mub[:], in_=mu[:])
            var = small.tile([C, 1], f32, tag="var")
            nc.vector.tensor_tensor(var[:], mu[:], mu[:], mult_op)
            nc.vector.tensor_tensor(var[:], psb[:, 1:2], var[:], sub_op)
            # wp needs no rstd: cast while ACT computes rstd
            w_p = wgt.tile([C, C], bf16, tag="wp")
            nc.vector.tensor_copy(out=w_p[:], in_=w_p0[:])
            lnv = small.tile([C, 1], f32, tag="lnv")
            nc.scalar.activation(out=lnv[:], in_=var[:], func=Ln, bias=eps_sb[:])
            rstd = small.tile([C, 1], f32, tag="rstd")
            nc.scalar.activation(out=rstd[:], in_=lnv[:], func=Exp, scale=-0.5)

            # fold rstd into the bf16 weights (per-in-channel scale)
            w_q = wgt.tile([C, C], bf16, tag="wq")
            nc.vector.tensor_scalar(w_q[:], w_q0[:], rstd[:], None, op0=mult_op)
            w_k = wgt.tile([C, C], bf16, tag="wk")
            nc.vector.tensor_scalar(w_k[:], w_k0[:], rstd[:], None, op0=mult_op)
            w_v = wgt.tile([C, C], bf16, tag="wv")
            nc.vector.tensor_scalar(w_v[:], w_v0[:], rstd[:], None, op0=mult_op)

            def xbpart(lo, width):
                j = lo // CHW
                assert lo + width <= (j + 1) * CHW
                return xb[j][:, lo - j * CHW : lo - j * CHW + width]

            kc = [None] * 8

            def kpart(mi):
                return kc[mi // 4][:, (mi % 4) * 128 : (mi % 4) * 128 + 128]

            qb = [None] * NBLK
            vT_sb = big.tile([128, MT, C], bf16, tag="vt")
            bqs = small.tile([C, 1], f32, tag="bqs")
            bvb = small.tile([C, 1], bf16, tag="bvb")
            obs = small.tile([C, 1], f32, tag="obs")

            # --- critical-path projections: q0 and k slices 0,1 via ps_s ---
            psq0 = ps_s.tile([C, EXP_GRP, NB], f32, tag="s", name="psq0")
            nc.tensor.matmul(
                psq0[:, 0, :], lhsT=w_q[:], rhs=xbpart(0, NB), start=True, stop=True
            )
            # q-bias correction -Wq'.mu (reuses the wq stationary)
            psbq = ps_m.tile([C, 1], f32, tag="m")
            nc.tensor.matmul(psbq[:], lhsT=w_q[:], rhs=mub[:], start=True, stop=True)
            psk0 = ps_s.tile([C, EXP_GRP, NB], f32, tag="s", name="psk0")
            nc.tensor.matmul(
                psk0[:, 0, :], lhsT=w_k[:], rhs=xbpart(0, NB), start=True, stop=True
            )
            psk1 = ps_s.tile([C, EXP_GRP, NB], f32, tag="s", name="psk1")
            nc.tensor.matmul(
                psk1[:, 0, :], lhsT=w_k[:], rhs=xbpart(NB, NB), start=True, stop=True
            )
            # v-bias chain part 1: Wv'.mu
            psbv = ps_m.tile([C, 1], f32, tag="m")
            nc.tensor.matmul(psbv[:], lhsT=w_v[:], rhs=mub[:], start=True, stop=True)

            nc.vector.tensor_copy(out=bqs[:], in_=psbq[:])
            q0 = big.tile([C, NB], bf16, tag="q0")
            if use_bq:
                nc.vector.tensor_scalar(
                    q0[:], psq0[:, 0, :], bqs[:], bqe_sb[:], op0=sub_op, op1=add_op
                )
            else:
                nc.vector.tensor_scalar(q0[:], psq0[:, 0, :], bqs[:], None, op0=sub_op)
            qb[0] = q0
            # k slice 0 on the scalar engine (idle until the first exp) so it
            # runs parallel to the q0 copy; slice 1 on DVE
            k0t = big.tile([C, NB], bf16, tag="k0")
            nc.scalar.activation(out=k0t[:], in_=psk0[:, 0, :], func=Copy)
            kc[0] = k0t
            k1t = big.tile([C, NB], bf16, tag="k1")
            nc.vector.tensor_copy(out=k1t[:], in_=psk1[:, 0, :])
            kc[1] = k1t
            nc.vector.tensor_copy(out=bvb[:], in_=psbv[:])

            # --- filler emitters (interleaved projections) ---
            def emit_k(s):
                psk = ps_m.tile([C, NB], f32, tag="m", name=f"psk{s}")
                nc.tensor.matmul(
                    psk[:], lhsT=w_k[:], rhs=xbpart(s * NB, NB), start=True, stop=True
                )
                kj = big.tile([C, NB], bf16, tag=f"k{s}")
                nc.vector.tensor_copy(out=kj[:], in_=psk[:])
                kc[s] = kj

            def emit_q(jb):
                psq = ps_m.tile([C, NB], f32, tag="m", name=f"psq{jb}")
                nc.tensor.matmul(
                    psq[:], lhsT=w_q[:], rhs=xbpart(jb * NB, NB), start=True, stop=True
                )
                qj = big.tile([C, NB], bf16, tag=f"q{jb}")
                if use_bq:
                    nc.vector.tensor_scalar(
                        qj[:], psq[:], bqs[:], bqe_sb[:], op0=sub_op, op1=add_op
                    )
                else:
                    nc.vector.tensor_scalar(qj[:], psq[:], bqs[:], None, op0=sub_op)
                qb[jb] = qj

            def emit_vb(b):
                # 4 transposed v tiles into one psum bank, one DVE drain
                psv = ps_m.tile([128, 4, C], f32, tag="m", name=f"psv{b}")
                for t in range(4):
                    mi = 4 * b + t
                    nc.tensor.matmul(
                        psv[:, t, :],
                        lhsT=xbpart(mi * 128, 128),
                        rhs=w_v[:],
                        start=True,
                        stop=True,
                    )
                nc.vector.tensor_copy(out=vT_sb[:, 4 * b : 4 * b + 4, :], in_=psv[:])

            def emit_xb3():
                nc.vector.tensor_copy(out=xb3[:], in_=xc[3][:])

            def emit_psob():
                # v-bias chain part 2: obias = Wp.(Wv'.mu)  [minus bpe if any]
                psob = ps_m.tile([C, 1], f32, tag="m")
                nc.tensor.matmul(psob[:], lhsT=w_p[:], rhs=bvb[:], start=True, stop=True)
                if use_bp:
                    nc.vector.tensor_tensor(obs[:], psob[:], bpe_sb[:], sub_op)
                else:
                    nc.vector.tensor_copy(out=obs[:], in_=psob[:])

            # pre-loop fillers (ps_m FIFO paces one copy per slot)
            emit_k(2)
            emit_vb(0)

            # filler schedule: (block, group) -> emitter.  k slice s must land
            # before score group 2s; v batch b before PV step 2b+1; q_jb
            # before block jb; psob before block 0's (deferred) tail.
            fillers = {
                (0, 0): lambda: emit_k(3),
                (0, 1): lambda: (emit_vb(1), emit_xb3()),
                (0, 2): lambda: emit_k(4),
                (0, 3): lambda: emit_vb(2),
                (0, 4): lambda: emit_vb(3),
                (0, 5): lambda: emit_vb(4),
                (0, 6): lambda: emit_k(5),
                (0, 7): lambda: emit_vb(5),
                (0, 8): lambda: emit_q(1),
                (0, 9): lambda: emit_vb(6),
                (0, 10): lambda: emit_k(6),
                (0, 11): lambda: emit_vb(7),
                (0, 12): lambda: emit_k(7),
                (0, 14): emit_psob,
                (1, 0): lambda: emit_q(2),
                (1, 1): lambda: emit_q(3),
            }

            # deferred tail state: (pv, dn, dacc_fold, jb) of the previous block
            pend = [None]

            def tail_a():
                # den fold matmul + reciprocal of the previous block; if the
                # whole denominator accumulated on DVE this is dn's only
                # matmul and must open the psum accumulation group
                pv_p, dn_p, dfold_p, jb_p, den_g_p = pend[0]
                nc.tensor.matmul(
                    dn_p[:], lhsT=ones_bf[:], rhs=dfold_p[:],
                    start=(den_g_p == NG), stop=True,
                )
                rden = ostage.tile([128, NB], f32, tag="rden")
                nc.vector.reciprocal_approx_fast(out=rden[:], in_=dn_p[:])
                pend[0] = (pv_p, rden, jb_p)

            def tail_b():
                # PV drain, output projection, residual, store
                pv_p, rden, jb_p = pend[0]
                pend[0] = None
                hv = ostage.tile([C, NB], bf16, tag="hv")
                nc.vector.tensor_tensor(hv[:], pv_p[:], rden[:], mult_op)
                pso = ps_m.tile([C, NB], f32, tag="m")
                nc.tensor.matmul(pso[:], lhsT=w_p[:], rhs=hv[:], start=True, stop=True)
                o1 = ostage.tile([C, NB], f32, tag="o1")
                xblk = xc[jb_p // 2][:, (jb_p % 2) * 512 : (jb_p % 2) * 512 + 512]
                nc.vector.scalar_tensor_tensor(
                    o1[:], pso[:], obs[:], xblk, op0=sub_op, op1=add_op
                )
                nc.sync.dma_start(
                    out=out_d[:, jb_p * NB : (jb_p + 1) * NB], in_=o1[:]
                )

            # --- attention over query blocks ---
            for jb in range(NBLK):
                den_g = DEN_SPLIT[jb]
                qs = qb[jb][:]
                pv = ps_pv.tile([C, NB], f32, tag="pv")
                dn = ps_den.tile([C, NB], f32, tag="dn")
                dacc = ostage.tile([128, EXP_GRP, NB], bf16, tag="dacc")
                dfold = ostage.tile([128, NB], bf16, tag="dfold")
                pts = [None] * NG
                # software-pipelined by one group: scores/exp for g are
                # emitted ahead of group g-1's consumers so the scalar engine
                # never starves behind PV/den matmuls.
                for g in range(NG + 1):
                    if g < NG:
                        ss = ps_s.tile([128, EXP_GRP, NB], f32, tag="s")
                        for u in range(EXP_GRP):
                            mi = g * EXP_GRP + u
                            nc.tensor.matmul(
                                ss[:, u, :],
                                lhsT=kpart(mi),
                                rhs=qs,
                                start=True,
                                stop=True,
                            )
                        pt = ptile.tile([128, EXP_GRP, NB], bf16, tag="pt")
                        nc.scalar.activation(
                            out=pt[:], in_=ss[:], func=Exp, scale=SCALE
                        )
                        pts[g] = pt
                        fill = fillers.get((jb, g))
                        if fill is not None:
                            fill()
                    if pend[0] is not None:
                        if g == 2:
                            tail_a()
                        elif g == 4:
                            tail_b()
                    if g == 0:
                        continue
                    c = g - 1
                    pt = pts[c]
                    pts[c] = None
                    for u in range(EXP_GRP):
                        mi = c * EXP_GRP + u
                        nc.tensor.matmul(
                            pv[:],
                            lhsT=vT_sb[:, mi, :],
                            rhs=pt[:, u, :],
                            start=(mi == 0),
                            stop=(mi == MT - 1),
                        )
                    if c < den_g:
                        # denominator partial on DVE (bf16 SBUF adds, 2x mode)
                        if c == 0:
                            nc.vector.tensor_copy(out=dacc[:], in_=pt[:])
                        else:
                            nc.vector.tensor_tensor(dacc[:], dacc[:], pt[:], add_op)
                        if c == den_g - 1:
                            nc.vector.tensor_tensor(
                                dfold[:], dacc[:, 0, :], dacc[:, 1, :], add_op
                            )
                    else:
                        # denominator partial on PE; the all-ones stationary
                        # writes den into every output partition, so the
                        # reciprocal needs no cross-partition broadcast
                        for u in range(EXP_GRP):
                            mi = c * EXP_GRP + u
                            nc.tensor.matmul(
                                dn[:],
                                lhsT=ones_bf[:],
                                rhs=pt[:, u, :],
                                start=(c == den_g and u == 0),
                                stop=False,
                            )
                pend[0] = (pv, dn, dfold, jb, den_g)
            # last block's tail runs immediately
            tail_a()
            tail_b()

    nc.compile()
    _NC_CACHE[key] = nc
    return nc


def kernel(**inputs):
    global LAST_RESULTS
    _install_ntff_hook()
    from concourse.bass_utils import run_bass_kernel_spmd

    ins = {
        k: np.ascontiguousarray(np.asarray(v), dtype=np.float32)
        for k, v in inputs.items()
    }
    x = ins["x"]
    gs, gb = ins["gn_scale"], ins["gn_bias"]

    # Fold the GroupNorm affine into the q/k/v weights; pre-transpose all
    # weights into the [in_channel, out_channel] layout the PE wants.
    wq_e = ins["wq"] * gs[None, :]
    wk_e = ins["wk"] * gs[None, :]
    wv_e = ins["wv"] * gs[None, :]
    wqT = np.ascontiguousarray(wq_e.T)
    wkT = np.ascontiguousarray(wk_e.T)
    wvT = np.ascontiguousarray(wv_e.T)
    wpT = np.ascontiguousarray(ins["wp"].T)
    bq_e = (ins["bq"] + ins["wq"] @ gb).reshape(C, 1)
    bv_e = ins["bv"] + ins["wv"] @ gb
    bp_e = (ins["bp"] + ins["wp"] @ bv_e).reshape(C, 1)
    use_bq = bool(np.any(bq_e))
    use_bp = bool(np.any(bp_e))

    nc = _build(use_bq, use_bp)

    in_maps = []
    for core in range(8):
        b, half = core // 2, core % 2
        xb = x[b].reshape(C, N)
        if half == 1:
            xb = np.concatenate([xb[:, NQ:], xb[:, :NQ]], axis=1)
        in_maps.append(
            {
                "xp": np.ascontiguousarray(xb),
                "wqT": wqT,
                "wkT": wkT,
                "wvT": wvT,
                "wpT": wpT,
                "bqe": bq_e,
                "bpe": bp_e,
            }
        )

    trace = os.environ.get("KERNEL_TRACE", "0") == "1"
    res = run_bass_kernel_spmd(nc, in_maps, core_ids=list(range(8)), trace=trace)
    LAST_RESULTS = res

    out = np.empty((B, C, N), np.float32)
    for core in range(8):
        b, half = core // 2, core % 2
        out[b, :, half * NQ : (half + 1) * NQ] = res.results[core]["out"]
    return out.reshape(B, C, H, W)
